# revision 7
# baseline (speedup 1.0000x reference)
"""MemoryBank MoE-routing kernel for 8 Trainium2 NeuronCores.

Reference semantics (B=16, S=2048, D=1024, M=512, T=256, K=8):
    x0 = x[:, 0, :]                          # [B, D]
    scores = x0 @ memory_router              # [B, M]
    top_vals, top_idx = top_k(scores, 8)     # [B, K]
    w = softmax(top_vals)                    # [B, K]
    combined = sum_k w[b,k] * memory_tokens[top_idx[b,k]]   # [B, T, D]
    out = x;  out[:, 1:T+1, :] = combined

Sharding: data-parallel over batch (2 batches per core), memory_tokens and
memory_router replicated on every core.  Each core does its own routing,
gathers its 16 selected memory rows with register-offset DMAs (each expert row
is a contiguous 1 MiB block), accumulates with DVE fused multiply-add, and
writes its full [2, S, D] output slice (pass-through rows copied DRAM->DRAM).
No collectives needed.
"""

import numpy as np

import concourse.bass as bass
import concourse.bacc as bacc
import concourse.mybir as mybir
from concourse import tile
from concourse.bass_utils import run_bass_kernel_spmd

N_CORES = 8
B, S, D = 16, 2048, 1024
M, T = 512, 256
K = 8
B_LOC = B // N_CORES  # batches per core
KT = D // 128         # contraction tiles for the router matmul

F32 = mybir.dt.float32
U32 = mybir.dt.uint32


def build_program():
    nc = bacc.Bacc(
        "TRN2",
        target_bir_lowering=False,
        debug=False,
        enable_asserts=True,
        num_devices=N_CORES,
    )

    x = nc.dram_tensor("x", [B_LOC, S, D], F32, kind="ExternalInput")
    mem = nc.dram_tensor("mem", [M, T, D], F32, kind="ExternalInput")
    router = nc.dram_tensor("router", [D, M], F32, kind="ExternalInput")
    out = nc.dram_tensor("out", [B_LOC, S, D], F32, kind="ExternalOutput")

    with tile.TileContext(nc) as tc:
        with (
            tc.tile_pool(name="sbuf", bufs=1) as sp,
            tc.tile_pool(name="gpool", bufs=4) as gp,
            tc.tile_pool(name="psum", bufs=1, space="PSUM") as pp,
            tc.tile_pool(name="dram", bufs=1, space="DRAM") as dp,
        ):
            # ---- pass-through copy of the rows the module doesn't touch ----
            nc.sync.dma_start(out=out[:, 0, :], in_=x[:, 0, :])
            for b in range(B_LOC):
                nc.sync.dma_start(out=out[b, T + 1 : S, :], in_=x[b, T + 1 : S, :])

            # ---- router scores: [B_LOC, M] = x0 @ router ----
            wt = sp.tile([128, KT * M], F32)  # router as (p, kt, m)
            nc.sync.dma_start(
                out=wt[:].rearrange("p (kt m) -> p kt m", kt=KT),
                in_=router[:, :].rearrange("(kt p) m -> p kt m", p=128),
            )
            x0t = sp.tile([128, B_LOC * KT], F32)  # x0^T as (p, b, kt)
            for b in range(B_LOC):
                nc.sync.dma_start(
                    out=x0t[:, b * KT : (b + 1) * KT],
                    in_=x[b, 0, :].rearrange("(kt p) -> p kt", p=128),
                )
            x0t_v = x0t[:].rearrange("p (b kt) -> p b kt", b=B_LOC)

            scores_p = pp.tile([B_LOC, M], F32)
            for kt in range(KT):
                nc.tensor.matmul(
                    out=scores_p[:],
                    lhsT=x0t_v[:, :, kt],
                    rhs=wt[:, kt * M : (kt + 1) * M],
                    start=(kt == 0),
                    stop=(kt == KT - 1),
                )
            scores = sp.tile([B_LOC, M], F32)
            nc.vector.tensor_copy(out=scores[:], in_=scores_p[:])

            # ---- top-8 + softmax ----
            vals = sp.tile([B_LOC, K], F32)
            nc.vector.max(out=vals[:], in_=scores[:])
            idx = sp.tile([B_LOC, K], U32)
            nc.vector.max_index(out=idx[:], in_max=vals[:], in_values=scores[:])

            negmax = sp.tile([B_LOC, 1], F32)
            nc.vector.tensor_scalar_mul(negmax[:], vals[:, 0:1], -1.0)
            ex = sp.tile([B_LOC, K], F32)
            ssum = sp.tile([B_LOC, 1], F32)
            nc.scalar.activation(
                out=ex[:],
                in_=vals[:],
                func=mybir.ActivationFunctionType.Exp,
                bias=negmax[:, 0:1],
                scale=1.0,
                accum_out=ssum[:, 0:1],
            )
            rec = sp.tile([B_LOC, 1], F32)
            nc.vector.reciprocal(rec[:], ssum[:])
            w = sp.tile([B_LOC, K], F32)
            nc.vector.tensor_scalar(
                out=w[:],
                in0=ex[:],
                scalar1=rec[:, 0:1],
                scalar2=None,
                op0=mybir.AluOpType.mult,
            )

            # ---- move idx/w to partition 0 (DRAM bounce) + broadcast ----
            w_dram = dp.tile([B_LOC, K], F32)
            idx_dram = dp.tile([B_LOC, K], U32)
            nc.sync.dma_start(out=w_dram[:], in_=w[:])
            nc.sync.dma_start(out=idx_dram[:], in_=idx[:])
            w0 = sp.tile([1, B_LOC * K], F32)
            i0 = sp.tile([1, B_LOC * K], U32)
            nc.sync.dma_start(out=w0[:], in_=w_dram[:].rearrange("b k -> (b k)"))
            nc.sync.dma_start(out=i0[:], in_=idx_dram[:].rearrange("b k -> (b k)"))
            w_bc = sp.tile([128, B_LOC * K], F32)
            nc.gpsimd.partition_broadcast(w_bc[:], w0[0:1, :])
            i0f = sp.tile([1, B_LOC * K], F32)
            nc.vector.tensor_copy(out=i0f[:], in_=i0[:])
            ibc = sp.tile([128, B_LOC * K], F32)
            nc.gpsimd.partition_broadcast(ibc[:], i0f[0:1, :])

            # ---- per-partition row indices into mem viewed as [(m t), d] ----
            # for (b,k) col and half h: row[p] = idx[b,k]*T + h*128 + p
            iota = sp.tile([128, 1], mybir.dt.int32)
            nc.gpsimd.iota(iota[:], pattern=[[0, 1]], base=0, channel_multiplier=1)
            iotaf = sp.tile([128, 1], F32)
            nc.vector.tensor_copy(out=iotaf[:], in_=iota[:])
            rid_u = []
            for h in range(2):
                ridf = sp.tile([128, B_LOC * K], F32, tag=f"ridf{h}")
                nc.vector.scalar_tensor_tensor(
                    out=ridf[:],
                    in0=ibc[:],
                    scalar=float(T),
                    in1=iotaf[:, 0:1].to_broadcast([128, B_LOC * K]),
                    op0=mybir.AluOpType.mult,
                    op1=mybir.AluOpType.add,
                )
                if h == 1:
                    nc.vector.tensor_scalar_add(ridf[:], ridf[:], 128.0)
                ru = sp.tile([128, B_LOC * K], U32, tag=f"ridu{h}")
                nc.vector.tensor_copy(out=ru[:], in_=ridf[:])
                rid_u.append(ru)

            # ---- gather selected experts + weighted accumulate ----
            # Expert row [T, D] viewed as [p=128, h=2, d=D] (t = h*128 + p).
            memflat = mem[:, :, :].rearrange("m t d -> (m t) d")
            cmbs = []
            for b in range(B_LOC):
                cmb = sp.tile([128, 2 * D], F32, tag=f"cmb{b}")
                cmbs.append(cmb)
                for k in range(K):
                    col = b * K + k
                    g = gp.tile([128, 2 * D], F32, tag="g")
                    for h in range(2):
                        nc.gpsimd.indirect_dma_start(
                            out=g[:, h * D : (h + 1) * D],
                            out_offset=None,
                            in_=memflat,
                            in_offset=bass.IndirectOffsetOnAxis(
                                ap=rid_u[h][:, col : col + 1], axis=0
                            ),
                        )
                    if k == 0:
                        nc.vector.tensor_scalar_mul(cmb[:], g[:], w_bc[:, col : col + 1])
                    else:
                        nc.vector.scalar_tensor_tensor(
                            out=cmb[:],
                            in0=g[:],
                            scalar=w_bc[:, col : col + 1],
                            in1=cmb[:],
                            op0=mybir.AluOpType.mult,
                            op1=mybir.AluOpType.add,
                        )

            # ---- write combined into rows 1..T of the output ----
            for b in range(B_LOC):
                nc.sync.dma_start(
                    out=out[b, 1 : T + 1, :].rearrange("(h p) d -> p h d", p=128),
                    in_=cmbs[b][:].rearrange("p (h d) -> p h d", h=2),
                )

    nc.compile()
    return nc


def kernel(x, memory_tokens, memory_router):
    nc = build_program()
    in_maps = [
        {
            "x": np.ascontiguousarray(x[c * B_LOC : (c + 1) * B_LOC]),
            "mem": memory_tokens,
            "router": memory_router,
        }
        for c in range(N_CORES)
    ]
    res = run_bass_kernel_spmd(nc, in_maps, list(range(N_CORES)))
    return np.concatenate(
        [res.results[c]["out"] for c in range(N_CORES)], axis=0
    )


# revision 8
# speedup vs baseline: 1.0942x; 1.0942x over previous
"""MemoryBank MoE-routing kernel for 8 Trainium2 NeuronCores.

Reference semantics (B=16, S=2048, D=1024, M=512, T=256, K=8):
    x0 = x[:, 0, :]                          # [B, D]
    scores = x0 @ memory_router              # [B, M]
    top_vals, top_idx = top_k(scores, 8)     # [B, K]
    w = softmax(top_vals)                    # [B, K]
    combined = sum_k w[b,k] * memory_tokens[top_idx[b,k]]   # [B, T, D]
    out = x;  out[:, 1:T+1, :] = combined

Sharding: data-parallel over batch (2 batches per core), memory_tokens and
memory_router replicated on every core.  Each core does its own routing
(PE matmul -> DVE max/max_index -> softmax), gathers its 16 selected memory
rows with indirect DMAs (memory viewed as [M*128, 2*D] so each descriptor
moves two contiguous t-rows = 8 KiB), accumulates with DVE fused
multiply-add, and writes its full [2, S, D] output slice.  The pass-through
rows are copied DRAM->DRAM on the ACT HWDGE ring so they never block the
small routing DMAs on the SP ring.  No collectives needed.
"""

import numpy as np

import concourse.bass as bass
import concourse.bacc as bacc
import concourse.mybir as mybir
from concourse import tile
from concourse.bass_utils import run_bass_kernel_spmd

N_CORES = 8
B, S, D = 16, 2048, 1024
M, T = 512, 256
K = 8
B_LOC = B // N_CORES  # batches per core
KT = D // 128         # contraction tiles for the router matmul

F32 = mybir.dt.float32
U32 = mybir.dt.uint32


def build_program():
    nc = bacc.Bacc(
        "TRN2",
        target_bir_lowering=False,
        debug=False,
        enable_asserts=True,
        num_devices=N_CORES,
    )

    x = nc.dram_tensor("x", [B_LOC, S, D], F32, kind="ExternalInput")
    mem = nc.dram_tensor("mem", [M, T, D], F32, kind="ExternalInput")
    router = nc.dram_tensor("router", [D, M], F32, kind="ExternalInput")
    out = nc.dram_tensor("out", [B_LOC, S, D], F32, kind="ExternalOutput")

    with tile.TileContext(nc) as tc:
        with (
            tc.tile_pool(name="sbuf", bufs=1) as sp,
            tc.tile_pool(name="gpool", bufs=6) as gp,
            tc.tile_pool(name="psum", bufs=1, space="PSUM") as pp,
            tc.tile_pool(name="dram", bufs=1, space="DRAM") as dp,
        ):
            # ---- routing inputs on the SP ring (issue first: critical path) ----
            wt = sp.tile([128, KT * M], F32)  # router as (p, kt, m)
            nc.sync.dma_start(
                out=wt[:].rearrange("p (kt m) -> p kt m", kt=KT),
                in_=router[:, :].rearrange("(kt p) m -> p kt m", p=128),
            )
            x0t = sp.tile([128, B_LOC * KT], F32)  # x0^T as (p, b, kt)
            for b in range(B_LOC):
                nc.sync.dma_start(
                    out=x0t[:, b * KT : (b + 1) * KT],
                    in_=x[b, 0, :].rearrange("(kt p) -> p kt", p=128),
                )
            x0t_v = x0t[:].rearrange("p (b kt) -> p b kt", b=B_LOC)

            # ---- pass-through copies on the ACT ring (big, independent) ----
            nc.scalar.dma_start(out=out[:, 0, :], in_=x[:, 0, :])
            for b in range(B_LOC):
                nc.scalar.dma_start(out=out[b, T + 1 : S, :], in_=x[b, T + 1 : S, :])

            # ---- router scores: [B_LOC, M] = x0 @ router (fp32 on PE) ----
            scores_p = pp.tile([B_LOC, M], F32)
            for kt in range(KT):
                nc.tensor.matmul(
                    out=scores_p[:],
                    lhsT=x0t_v[:, :, kt],
                    rhs=wt[:, kt * M : (kt + 1) * M],
                    start=(kt == 0),
                    stop=(kt == KT - 1),
                )

            # ---- top-8 + softmax (max/max_index read PSUM directly) ----
            vals = sp.tile([B_LOC, K], F32)
            nc.vector.max(out=vals[:], in_=scores_p[:])
            idx = sp.tile([B_LOC, K], U32)
            nc.vector.max_index(out=idx[:], in_max=vals[:], in_values=scores_p[:])

            negmax = sp.tile([B_LOC, 1], F32)
            nc.vector.tensor_scalar_mul(negmax[:], vals[:, 0:1], -1.0)
            ex = sp.tile([B_LOC, K], F32)
            ssum = sp.tile([B_LOC, 1], F32)
            nc.scalar.activation(
                out=ex[:],
                in_=vals[:],
                func=mybir.ActivationFunctionType.Exp,
                bias=negmax[:, 0:1],
                scale=1.0,
                accum_out=ssum[:, 0:1],
            )
            rec = sp.tile([B_LOC, 1], F32)
            nc.vector.reciprocal(rec[:], ssum[:])
            w = sp.tile([B_LOC, K], F32)
            nc.vector.tensor_scalar(
                out=w[:],
                in0=ex[:],
                scalar1=rec[:, 0:1],
                scalar2=None,
                op0=mybir.AluOpType.mult,
            )

            # ---- move idx/w to partition 0 (DRAM bounce) + broadcast ----
            w_dram = dp.tile([B_LOC, K], F32)
            idx_dram = dp.tile([B_LOC, K], U32)
            nc.sync.dma_start(out=w_dram[:], in_=w[:])
            nc.sync.dma_start(out=idx_dram[:], in_=idx[:])
            w0 = sp.tile([1, B_LOC * K], F32)
            i0 = sp.tile([1, B_LOC * K], U32)
            nc.sync.dma_start(out=w0[:], in_=w_dram[:].rearrange("b k -> (b k)"))
            nc.sync.dma_start(out=i0[:], in_=idx_dram[:].rearrange("b k -> (b k)"))
            w_bc = sp.tile([128, B_LOC * K], F32)
            nc.gpsimd.partition_broadcast(w_bc[:], w0[0:1, :])
            i0f = sp.tile([1, B_LOC * K], F32)
            nc.vector.tensor_copy(out=i0f[:], in_=i0[:])
            ibc = sp.tile([128, B_LOC * K], F32)
            nc.gpsimd.partition_broadcast(ibc[:], i0f[0:1, :])

            # ---- per-partition row indices into mem viewed [(m t2), (j d)] ----
            # for (b,k) col: row[p] = idx[b,k]*128 + p   (two t-rows per row)
            iota = sp.tile([128, 1], mybir.dt.int32)
            nc.gpsimd.iota(iota[:], pattern=[[0, 1]], base=0, channel_multiplier=1)
            iotaf = sp.tile([128, 1], F32)
            nc.vector.tensor_copy(out=iotaf[:], in_=iota[:])
            ridf = sp.tile([128, B_LOC * K], F32)
            nc.vector.scalar_tensor_tensor(
                out=ridf[:],
                in0=ibc[:],
                scalar=float(T // 2),
                in1=iotaf[:, 0:1].to_broadcast([128, B_LOC * K]),
                op0=mybir.AluOpType.mult,
                op1=mybir.AluOpType.add,
            )
            ridu = sp.tile([128, B_LOC * K], U32)
            nc.vector.tensor_copy(out=ridu[:], in_=ridf[:])

            # ---- gather selected experts + weighted accumulate ----
            # mem [M, T, D] viewed as [(m t2), (j d)]: row r = m*128 + t2 holds
            # t-rows 2*t2 and 2*t2+1 (8 KiB per descriptor).
            mem2 = mem[:, :, :].rearrange("m (t2 j) d -> (m t2) (j d)", j=2)
            cmbs = []
            for b in range(B_LOC):
                cmb = sp.tile([128, 2 * D], F32, tag=f"cmb{b}")
                cmbs.append(cmb)
                for k in range(K):
                    col = b * K + k
                    g = gp.tile([128, 2 * D], F32, tag="g")
                    nc.gpsimd.indirect_dma_start(
                        out=g[:],
                        out_offset=None,
                        in_=mem2,
                        in_offset=bass.IndirectOffsetOnAxis(
                            ap=ridu[:, col : col + 1], axis=0
                        ),
                    )
                    if k == 0:
                        nc.vector.tensor_scalar_mul(cmb[:], g[:], w_bc[:, col : col + 1])
                    else:
                        nc.vector.scalar_tensor_tensor(
                            out=cmb[:],
                            in0=g[:],
                            scalar=w_bc[:, col : col + 1],
                            in1=cmb[:],
                            op0=mybir.AluOpType.mult,
                            op1=mybir.AluOpType.add,
                        )

            # ---- write combined into rows 1..T of the output ----
            # cmb[p, (j d)] holds t-rows t = 2*p + j.
            for b in range(B_LOC):
                nc.sync.dma_start(
                    out=out[b, 1 : T + 1, :].rearrange("(p j) d -> p j d", j=2),
                    in_=cmbs[b][:].rearrange("p (j d) -> p j d", j=2),
                )

    nc.compile()
    return nc


def kernel(x, memory_tokens, memory_router):
    nc = build_program()
    in_maps = [
        {
            "x": np.ascontiguousarray(x[c * B_LOC : (c + 1) * B_LOC]),
            "mem": memory_tokens,
            "router": memory_router,
        }
        for c in range(N_CORES)
    ]
    res = run_bass_kernel_spmd(nc, in_maps, list(range(N_CORES)))
    return np.concatenate(
        [res.results[c]["out"] for c in range(N_CORES)], axis=0
    )


# revision 9
# speedup vs baseline: 1.3393x; 1.2239x over previous
"""MemoryBank MoE-routing kernel for 8 Trainium2 NeuronCores.

Reference semantics (B=16, S=2048, D=1024, M=512, T=256, K=8):
    x0 = x[:, 0, :]                          # [B, D]
    scores = x0 @ memory_router              # [B, M]
    top_vals, top_idx = top_k(scores, 8)     # [B, K]
    w = softmax(top_vals)                    # [B, K]
    combined = sum_k w[b,k] * memory_tokens[top_idx[b,k]]   # [B, T, D]
    out = x;  out[:, 1:T+1, :] = combined

Sharding: data-parallel over batch (2 batches per core), memory_tokens and
memory_router replicated on every core.  Each core does its own routing
(PE matmul -> DVE max/max_index -> softmax), gathers its 16 selected memory
rows with indirect DMAs (memory viewed as [M*128, 2*D] so each descriptor
moves two contiguous t-rows = 8 KiB), accumulates with DVE fused
multiply-add, and writes its full [2, S, D] output slice.  The pass-through
rows are copied DRAM->DRAM on the ACT HWDGE ring so they never block the
small routing DMAs on the SP ring.  No collectives needed.
"""

import numpy as np

import concourse.bass as bass
import concourse.bacc as bacc
import concourse.mybir as mybir
from concourse import tile
from concourse.bass_utils import run_bass_kernel_spmd

N_CORES = 8
B, S, D = 16, 2048, 1024
M, T = 512, 256
K = 8
B_LOC = B // N_CORES  # batches per core
KT = D // 128         # contraction tiles for the router matmul

F32 = mybir.dt.float32
U32 = mybir.dt.uint32


def build_program():
    nc = bacc.Bacc(
        "TRN2",
        target_bir_lowering=False,
        debug=False,
        enable_asserts=True,
        num_devices=N_CORES,
    )

    x = nc.dram_tensor("x", [B_LOC, S, D], F32, kind="ExternalInput")
    mem = nc.dram_tensor("mem", [M, T, D], F32, kind="ExternalInput")
    router = nc.dram_tensor("router", [D, M], F32, kind="ExternalInput")
    out = nc.dram_tensor("out", [B_LOC, S, D], F32, kind="ExternalOutput")

    with tile.TileContext(nc) as tc:
        with (
            tc.tile_pool(name="sbuf", bufs=1) as sp,
            tc.tile_pool(name="gpool", bufs=6) as gp,
            tc.tile_pool(name="psum", bufs=1, space="PSUM") as pp,
            tc.tile_pool(name="dram", bufs=1, space="DRAM") as dp,
        ):
            # ---- routing inputs on the SP ring (issue first: critical path) ----
            wt = sp.tile([128, KT * M], F32)  # router as (p, kt, m)
            nc.sync.dma_start(
                out=wt[:].rearrange("p (kt m) -> p kt m", kt=KT),
                in_=router[:, :].rearrange("(kt p) m -> p kt m", p=128),
            )
            x0t = sp.tile([128, B_LOC * KT], F32)  # x0^T as (p, b, kt)
            for b in range(B_LOC):
                nc.sync.dma_start(
                    out=x0t[:, b * KT : (b + 1) * KT],
                    in_=x[b, 0, :].rearrange("(kt p) -> p kt", p=128),
                )
            x0t_v = x0t[:].rearrange("p (b kt) -> p b kt", b=B_LOC)

            # ---- pass-through copies: same SP ring, AFTER the routing loads.
            # HWDGE drains one ring in FIFO order, so the small critical-path
            # loads above complete at full rate before these 28 MiB start.
            nc.sync.dma_start(out=out[:, 0, :], in_=x[:, 0, :])
            for b in range(B_LOC):
                nc.sync.dma_start(out=out[b, T + 1 : S, :], in_=x[b, T + 1 : S, :])

            # ---- router scores: [B_LOC, M] = x0 @ router (fp32 on PE) ----
            scores_p = pp.tile([B_LOC, M], F32)
            for kt in range(KT):
                nc.tensor.matmul(
                    out=scores_p[:],
                    lhsT=x0t_v[:, :, kt],
                    rhs=wt[:, kt * M : (kt + 1) * M],
                    start=(kt == 0),
                    stop=(kt == KT - 1),
                )

            # ---- top-8 + softmax (max/max_index read PSUM directly) ----
            vals = sp.tile([B_LOC, K], F32)
            nc.vector.max(out=vals[:], in_=scores_p[:])
            idx = sp.tile([B_LOC, K], U32)
            nc.vector.max_index(out=idx[:], in_max=vals[:], in_values=scores_p[:])

            negmax = sp.tile([B_LOC, 1], F32)
            nc.vector.tensor_scalar_mul(negmax[:], vals[:, 0:1], -1.0)
            ex = sp.tile([B_LOC, K], F32)
            ssum = sp.tile([B_LOC, 1], F32)
            nc.scalar.activation(
                out=ex[:],
                in_=vals[:],
                func=mybir.ActivationFunctionType.Exp,
                bias=negmax[:, 0:1],
                scale=1.0,
                accum_out=ssum[:, 0:1],
            )
            rec = sp.tile([B_LOC, 1], F32)
            nc.vector.reciprocal(rec[:], ssum[:])
            w = sp.tile([B_LOC, K], F32)
            nc.vector.tensor_scalar(
                out=w[:],
                in0=ex[:],
                scalar1=rec[:, 0:1],
                scalar2=None,
                op0=mybir.AluOpType.mult,
            )

            # ---- move idx/w to partition 0 (DRAM bounce) + broadcast ----
            w_dram = dp.tile([B_LOC, K], F32)
            idx_dram = dp.tile([B_LOC, K], U32)
            nc.sync.dma_start(out=w_dram[:], in_=w[:])
            nc.sync.dma_start(out=idx_dram[:], in_=idx[:])
            w0 = sp.tile([1, B_LOC * K], F32)
            i0 = sp.tile([1, B_LOC * K], U32)
            nc.sync.dma_start(out=w0[:], in_=w_dram[:].rearrange("b k -> (b k)"))
            nc.sync.dma_start(out=i0[:], in_=idx_dram[:].rearrange("b k -> (b k)"))
            w_bc = sp.tile([128, B_LOC * K], F32)
            nc.gpsimd.partition_broadcast(w_bc[:], w0[0:1, :])
            i0f = sp.tile([1, B_LOC * K], F32)
            nc.vector.tensor_copy(out=i0f[:], in_=i0[:])
            ibc = sp.tile([128, B_LOC * K], F32)
            nc.gpsimd.partition_broadcast(ibc[:], i0f[0:1, :])

            # ---- per-partition row indices into mem viewed [(m t2), (j d)] ----
            # for (b,k) col: row[p] = idx[b,k]*128 + p   (two t-rows per row)
            iota = sp.tile([128, 1], mybir.dt.int32)
            nc.gpsimd.iota(iota[:], pattern=[[0, 1]], base=0, channel_multiplier=1)
            iotaf = sp.tile([128, 1], F32)
            nc.vector.tensor_copy(out=iotaf[:], in_=iota[:])
            ridf = sp.tile([128, B_LOC * K], F32)
            nc.vector.scalar_tensor_tensor(
                out=ridf[:],
                in0=ibc[:],
                scalar=float(T // 2),
                in1=iotaf[:, 0:1].to_broadcast([128, B_LOC * K]),
                op0=mybir.AluOpType.mult,
                op1=mybir.AluOpType.add,
            )
            ridu = sp.tile([128, B_LOC * K], U32)
            nc.vector.tensor_copy(out=ridu[:], in_=ridf[:])

            # ---- gather selected experts + weighted accumulate ----
            # mem [M, T, D] viewed as [(m t2), (j d)]: row r = m*128 + t2 holds
            # t-rows 2*t2 and 2*t2+1 (8 KiB per descriptor).
            mem2 = mem[:, :, :].rearrange("m (t2 j) d -> (m t2) (j d)", j=2)
            cmbs = []
            for b in range(B_LOC):
                cmb = sp.tile([128, 2 * D], F32, tag=f"cmb{b}")
                cmbs.append(cmb)
                for k in range(K):
                    col = b * K + k
                    g = gp.tile([128, 2 * D], F32, tag="g")
                    nc.gpsimd.indirect_dma_start(
                        out=g[:],
                        out_offset=None,
                        in_=mem2,
                        in_offset=bass.IndirectOffsetOnAxis(
                            ap=ridu[:, col : col + 1], axis=0
                        ),
                    )
                    if k == 0:
                        nc.vector.tensor_scalar_mul(cmb[:], g[:], w_bc[:, col : col + 1])
                    else:
                        nc.vector.scalar_tensor_tensor(
                            out=cmb[:],
                            in0=g[:],
                            scalar=w_bc[:, col : col + 1],
                            in1=cmb[:],
                            op0=mybir.AluOpType.mult,
                            op1=mybir.AluOpType.add,
                        )

            # ---- write combined into rows 1..T of the output ----
            # cmb[p, (j d)] holds t-rows t = 2*p + j.
            for b in range(B_LOC):
                nc.sync.dma_start(
                    out=out[b, 1 : T + 1, :].rearrange("(p j) d -> p j d", j=2),
                    in_=cmbs[b][:].rearrange("p (j d) -> p j d", j=2),
                )

    nc.compile()
    return nc


def kernel(x, memory_tokens, memory_router):
    nc = build_program()
    in_maps = [
        {
            "x": np.ascontiguousarray(x[c * B_LOC : (c + 1) * B_LOC]),
            "mem": memory_tokens,
            "router": memory_router,
        }
        for c in range(N_CORES)
    ]
    res = run_bass_kernel_spmd(nc, in_maps, list(range(N_CORES)))
    return np.concatenate(
        [res.results[c]["out"] for c in range(N_CORES)], axis=0
    )


# revision 12
# speedup vs baseline: 1.4432x; 1.0776x over previous
"""MemoryBank MoE-routing kernel for 8 Trainium2 NeuronCores.

Reference semantics (B=16, S=2048, D=1024, M=512, T=256, K=8):
    x0 = x[:, 0, :]                          # [B, D]
    scores = x0 @ memory_router              # [B, M]
    top_vals, top_idx = top_k(scores, 8)     # [B, K]
    w = softmax(top_vals)                    # [B, K]
    combined = sum_k w[b,k] * memory_tokens[top_idx[b,k]]   # [B, T, D]
    out = x;  out[:, 1:T+1, :] = combined

Sharding: data-parallel over batch (2 batches per core), memory_tokens and
memory_router replicated on every core.  Each core does its own routing
(PE matmul -> DVE max/max_index -> softmax), gathers its 16 selected memory
rows with indirect DMAs (memory viewed as [M*128, 2*D] so each descriptor
moves two contiguous t-rows = 8 KiB), accumulates with DVE fused
multiply-add, and writes its full [2, S, D] output slice.  The pass-through
rows are copied DRAM->DRAM on the ACT HWDGE ring so they never block the
small routing DMAs on the SP ring.  No collectives needed.
"""

import numpy as np

import concourse.bass as bass
import concourse.bacc as bacc
import concourse.mybir as mybir
from concourse import tile
from concourse.bass_utils import run_bass_kernel_spmd

N_CORES = 8
B, S, D = 16, 2048, 1024
M, T = 512, 256
K = 8
B_LOC = B // N_CORES  # batches per core
KT = D // 128         # contraction tiles for the router matmul

F32 = mybir.dt.float32
U32 = mybir.dt.uint32


def build_program():
    nc = bacc.Bacc(
        "TRN2",
        target_bir_lowering=False,
        debug=False,
        enable_asserts=True,
        num_devices=N_CORES,
    )

    x = nc.dram_tensor("x", [B_LOC, S, D], F32, kind="ExternalInput")
    mem = nc.dram_tensor("mem", [M, T, D], F32, kind="ExternalInput")
    router = nc.dram_tensor("router", [D, M], F32, kind="ExternalInput")
    out = nc.dram_tensor("out", [B_LOC, S, D], F32, kind="ExternalOutput")

    with tile.TileContext(nc) as tc:
        with (
            tc.tile_pool(name="sbuf", bufs=1) as sp,
            tc.tile_pool(name="gpool", bufs=6) as gp,
            tc.tile_pool(name="psum", bufs=1, space="PSUM") as pp,
            tc.tile_pool(name="dram", bufs=1, space="DRAM") as dp,
        ):
            # ---- routing inputs on the SP ring (issue first: critical path) ----
            wt = sp.tile([128, KT * M], F32)  # router as (p, kt, m)
            nc.sync.dma_start(
                out=wt[:].rearrange("p (kt m) -> p kt m", kt=KT),
                in_=router[:, :].rearrange("(kt p) m -> p kt m", p=128),
            )
            x0t = sp.tile([128, B_LOC * KT], F32)  # x0^T as (p, b, kt)
            for b in range(B_LOC):
                nc.sync.dma_start(
                    out=x0t[:, b * KT : (b + 1) * KT],
                    in_=x[b, 0, :].rearrange("(kt p) -> p kt", p=128),
                )
            x0t_v = x0t[:].rearrange("p (b kt) -> p b kt", b=B_LOC)

            # ---- pass-through copies: same SP ring, AFTER the routing loads.
            # HWDGE drains one ring in FIFO order, so the small critical-path
            # loads above complete at full rate before these 28 MiB start.
            nc.sync.dma_start(out=out[:, 0, :], in_=x[:, 0, :])
            for b in range(B_LOC):
                nc.sync.dma_start(out=out[b, T + 1 : S, :], in_=x[b, T + 1 : S, :])

            # ---- router scores: [B_LOC, M] = x0 @ router (fp32 on PE) ----
            scores_p = pp.tile([B_LOC, M], F32)
            for kt in range(KT):
                nc.tensor.matmul(
                    out=scores_p[:],
                    lhsT=x0t_v[:, :, kt],
                    rhs=wt[:, kt * M : (kt + 1) * M],
                    start=(kt == 0),
                    stop=(kt == KT - 1),
                )

            # ---- top-8 + softmax (max/max_index read PSUM directly) ----
            vals = sp.tile([B_LOC, K], F32)
            nc.vector.max(out=vals[:], in_=scores_p[:])
            idx = sp.tile([B_LOC, K], U32)
            nc.vector.max_index(out=idx[:], in_max=vals[:], in_values=scores_p[:])

            negmax = sp.tile([B_LOC, 1], F32)
            nc.vector.tensor_scalar_mul(negmax[:], vals[:, 0:1], -1.0)
            ex = sp.tile([B_LOC, K], F32)
            ssum = sp.tile([B_LOC, 1], F32)
            nc.scalar.activation(
                out=ex[:],
                in_=vals[:],
                func=mybir.ActivationFunctionType.Exp,
                bias=negmax[:, 0:1],
                scale=1.0,
                accum_out=ssum[:, 0:1],
            )
            rec = sp.tile([B_LOC, 1], F32)
            nc.vector.reciprocal(rec[:], ssum[:])
            w = sp.tile([B_LOC, K], F32)
            nc.vector.tensor_scalar(
                out=w[:],
                in0=ex[:],
                scalar1=rec[:, 0:1],
                scalar2=None,
                op0=mybir.AluOpType.mult,
            )

            # ---- move idx/w to partition 0 (DRAM bounce) + broadcast ----
            # (ACT ring: the SP ring is busy draining the 28 MiB pass-through)
            w_dram = dp.tile([B_LOC, K], F32)
            idx_dram = dp.tile([B_LOC, K], U32)
            nc.scalar.dma_start(out=w_dram[:], in_=w[:])
            nc.scalar.dma_start(out=idx_dram[:], in_=idx[:])
            w0 = sp.tile([1, B_LOC * K], F32)
            i0 = sp.tile([1, B_LOC * K], U32)
            nc.scalar.dma_start(out=w0[:], in_=w_dram[:].rearrange("b k -> (b k)"))
            nc.scalar.dma_start(out=i0[:], in_=idx_dram[:].rearrange("b k -> (b k)"))
            w_bc = sp.tile([128, B_LOC * K], F32)
            nc.gpsimd.partition_broadcast(w_bc[:], w0[0:1, :])
            i0f = sp.tile([1, B_LOC * K], F32)
            nc.vector.tensor_copy(out=i0f[:], in_=i0[:])
            ibc = sp.tile([128, B_LOC * K], F32)
            nc.gpsimd.partition_broadcast(ibc[:], i0f[0:1, :])

            # ---- per-partition row indices into mem viewed [(m t2), (j d)] ----
            # for (b,k) col: row[p] = idx[b,k]*128 + p   (two t-rows per row)
            iota = sp.tile([128, 1], mybir.dt.int32)
            nc.gpsimd.iota(iota[:], pattern=[[0, 1]], base=0, channel_multiplier=1)
            iotaf = sp.tile([128, 1], F32)
            nc.vector.tensor_copy(out=iotaf[:], in_=iota[:])
            ridf = sp.tile([128, B_LOC * K], F32)
            nc.vector.scalar_tensor_tensor(
                out=ridf[:],
                in0=ibc[:],
                scalar=float(T // 2),
                in1=iotaf[:, 0:1].to_broadcast([128, B_LOC * K]),
                op0=mybir.AluOpType.mult,
                op1=mybir.AluOpType.add,
            )
            ridu = sp.tile([128, B_LOC * K], U32)
            nc.vector.tensor_copy(out=ridu[:], in_=ridf[:])

            # ---- gather selected experts + weighted accumulate ----
            # mem [M, T, D] viewed as [(m t2), (j d)]: row r = m*128 + t2 holds
            # t-rows 2*t2 and 2*t2+1 (8 KiB per descriptor).
            mem2 = mem[:, :, :].rearrange("m (t2 j) d -> (m t2) (j d)", j=2)
            cmbs = [
                sp.tile([128, 2 * D], F32, name=f"cmb{b}", tag=f"cmb{b}")
                for b in range(B_LOC)
            ]
            # interleave batches so both FMA chains progress concurrently
            for k in range(K):
                for b in range(B_LOC):
                    col = b * K + k
                    cmb = cmbs[b]
                    g = gp.tile([128, 2 * D], F32, tag="g")
                    nc.gpsimd.indirect_dma_start(
                        out=g[:],
                        out_offset=None,
                        in_=mem2,
                        in_offset=bass.IndirectOffsetOnAxis(
                            ap=ridu[:, col : col + 1], axis=0
                        ),
                    )
                    if k == 0:
                        nc.vector.tensor_scalar_mul(cmb[:], g[:], w_bc[:, col : col + 1])
                    else:
                        nc.vector.scalar_tensor_tensor(
                            out=cmb[:],
                            in0=g[:],
                            scalar=w_bc[:, col : col + 1],
                            in1=cmb[:],
                            op0=mybir.AluOpType.mult,
                            op1=mybir.AluOpType.add,
                        )

            # ---- write combined into rows 1..T of the output ----
            # cmb[p, (j d)] holds t-rows t = 2*p + j.  (ACT ring: idle by now.)
            for b in range(B_LOC):
                nc.scalar.dma_start(
                    out=out[b, 1 : T + 1, :].rearrange("(p j) d -> p j d", j=2),
                    in_=cmbs[b][:].rearrange("p (j d) -> p j d", j=2),
                )

    nc.compile()
    return nc


def kernel(x, memory_tokens, memory_router):
    nc = build_program()
    in_maps = [
        {
            "x": np.ascontiguousarray(x[c * B_LOC : (c + 1) * B_LOC]),
            "mem": memory_tokens,
            "router": memory_router,
        }
        for c in range(N_CORES)
    ]
    res = run_bass_kernel_spmd(nc, in_maps, list(range(N_CORES)))
    return np.concatenate(
        [res.results[c]["out"] for c in range(N_CORES)], axis=0
    )


# revision 14
# speedup vs baseline: 1.4767x; 1.0232x over previous
"""MemoryBank MoE-routing kernel for 8 Trainium2 NeuronCores.

Reference semantics (B=16, S=2048, D=1024, M=512, T=256, K=8):
    x0 = x[:, 0, :]                          # [B, D]
    scores = x0 @ memory_router              # [B, M]
    top_vals, top_idx = top_k(scores, 8)     # [B, K]
    w = softmax(top_vals)                    # [B, K]
    combined = sum_k w[b,k] * memory_tokens[top_idx[b,k]]   # [B, T, D]
    out = x;  out[:, 1:T+1, :] = combined

Sharding: data-parallel over batch (2 batches per core), memory_tokens and
memory_router replicated on every core.  Each core does its own routing
(PE matmul -> DVE max/max_index -> softmax), gathers its 16 selected memory
rows with indirect DMAs (memory viewed as [M*128, 2*D] so each descriptor
moves two contiguous t-rows = 8 KiB), accumulates with DVE fused
multiply-add, and writes its full [2, S, D] output slice.  The pass-through
rows are copied DRAM->DRAM on the ACT HWDGE ring so they never block the
small routing DMAs on the SP ring.  No collectives needed.
"""

import numpy as np

import concourse.bass as bass
import concourse.bacc as bacc
import concourse.mybir as mybir
from concourse import tile
from concourse.bass_utils import run_bass_kernel_spmd

N_CORES = 8
B, S, D = 16, 2048, 1024
M, T = 512, 256
K = 8
B_LOC = B // N_CORES  # batches per core
KT = D // 128         # contraction tiles for the router matmul

F32 = mybir.dt.float32
U32 = mybir.dt.uint32


def build_program():
    nc = bacc.Bacc(
        "TRN2",
        target_bir_lowering=False,
        debug=False,
        enable_asserts=True,
        num_devices=N_CORES,
    )

    x = nc.dram_tensor("x", [B_LOC, S, D], F32, kind="ExternalInput")
    mem = nc.dram_tensor("mem", [M, T, D], F32, kind="ExternalInput")
    router = nc.dram_tensor("router", [D, M], F32, kind="ExternalInput")
    out = nc.dram_tensor("out", [B_LOC, S, D], F32, kind="ExternalOutput")

    with tile.TileContext(nc) as tc:
        with (
            tc.tile_pool(name="sbuf", bufs=1) as sp,
            tc.tile_pool(name="gpool", bufs=6) as gp,
            tc.tile_pool(name="psum", bufs=1, space="PSUM") as pp,
            tc.tile_pool(name="dram", bufs=1, space="DRAM") as dp,
        ):
            # ---- routing inputs on the SP ring (issue first: critical path) ----
            wt = sp.tile([128, KT * M], F32)  # router as (p, kt, m)
            nc.sync.dma_start(
                out=wt[:].rearrange("p (kt m) -> p kt m", kt=KT),
                in_=router[:, :].rearrange("(kt p) m -> p kt m", p=128),
            )
            x0t = sp.tile([128, B_LOC * KT], F32)  # x0^T as (p, b, kt)
            for b in range(B_LOC):
                nc.sync.dma_start(
                    out=x0t[:, b * KT : (b + 1) * KT],
                    in_=x[b, 0, :].rearrange("(kt p) -> p kt", p=128),
                )
            x0t_v = x0t[:].rearrange("p (b kt) -> p b kt", b=B_LOC)

            # ---- pass-through copies: same SP ring, AFTER the routing loads.
            # HWDGE drains one ring in FIFO order, so the small critical-path
            # loads above complete at full rate before these 28 MiB start.
            nc.sync.dma_start(out=out[:, 0, :], in_=x[:, 0, :])
            for b in range(B_LOC):
                nc.sync.dma_start(out=out[b, T + 1 : S, :], in_=x[b, T + 1 : S, :])

            # ---- router scores, REPLICATED on all 128 partitions ----
            # lhsT column x0[b] broadcast to 128 stationary columns: every
            # PSUM partition row holds the same scores[b], so top-k/softmax
            # results are available on every partition with no broadcast step.
            iota = sp.tile([128, 1], mybir.dt.int32)
            nc.gpsimd.iota(iota[:], pattern=[[0, 1]], base=0, channel_multiplier=1)
            iotaf = sp.tile([128, 1], F32)
            nc.vector.tensor_copy(out=iotaf[:], in_=iota[:])

            w_all = []
            ridu_all = []
            for b in range(B_LOC):
                scores_p = pp.tile([128, M], F32, name=f"scores{b}", tag=f"scores{b}")
                for kt in range(KT):
                    nc.tensor.matmul(
                        out=scores_p[:],
                        lhsT=x0t_v[:, b : b + 1, kt].to_broadcast([128, 128]),
                        rhs=wt[:, kt * M : (kt + 1) * M],
                        start=(kt == 0),
                        stop=(kt == KT - 1),
                    )
                vals = sp.tile([128, K], F32, name=f"vals{b}", tag=f"vals{b}")
                nc.vector.max(out=vals[:], in_=scores_p[:])
                idx = sp.tile([128, K], U32, name=f"idx{b}", tag=f"idx{b}")
                nc.vector.max_index(out=idx[:], in_max=vals[:], in_values=scores_p[:])

                negmax = sp.tile([128, 1], F32, name=f"negmax{b}", tag=f"negmax{b}")
                nc.vector.tensor_scalar_mul(negmax[:], vals[:, 0:1], -1.0)
                ex = sp.tile([128, K], F32, name=f"ex{b}", tag=f"ex{b}")
                ssum = sp.tile([128, 1], F32, name=f"ssum{b}", tag=f"ssum{b}")
                nc.scalar.activation(
                    out=ex[:],
                    in_=vals[:],
                    func=mybir.ActivationFunctionType.Exp,
                    bias=negmax[:, 0:1],
                    scale=1.0,
                    accum_out=ssum[:, 0:1],
                )
                rec = sp.tile([128, 1], F32, name=f"rec{b}", tag=f"rec{b}")
                nc.vector.reciprocal(rec[:], ssum[:])
                w = sp.tile([128, K], F32, name=f"w{b}", tag=f"w{b}")
                nc.vector.tensor_scalar(
                    out=w[:],
                    in0=ex[:],
                    scalar1=rec[:, 0:1],
                    scalar2=None,
                    op0=mybir.AluOpType.mult,
                )
                w_all.append(w)

                # row indices into mem viewed [(m t2), (j d)]:
                # rid[p, k] = idx[b,k]*(T/2) + p   (two t-rows per row)
                idxf = sp.tile([128, K], F32, name=f"idxf{b}", tag=f"idxf{b}")
                nc.vector.tensor_copy(out=idxf[:], in_=idx[:])
                ridf = sp.tile([128, K], F32, name=f"ridf{b}", tag=f"ridf{b}")
                nc.vector.scalar_tensor_tensor(
                    out=ridf[:],
                    in0=idxf[:],
                    scalar=float(T // 2),
                    in1=iotaf[:, 0:1].to_broadcast([128, K]),
                    op0=mybir.AluOpType.mult,
                    op1=mybir.AluOpType.add,
                )
                ridu = sp.tile([128, K], U32, name=f"ridu{b}", tag=f"ridu{b}")
                nc.vector.tensor_copy(out=ridu[:], in_=ridf[:])
                ridu_all.append(ridu)

            # ---- gather selected experts + weighted accumulate ----
            # mem [M, T, D] viewed as [(m t2), (j d)]: row r = m*128 + t2 holds
            # t-rows 2*t2 and 2*t2+1 (8 KiB per descriptor).
            mem2 = mem[:, :, :].rearrange("m (t2 j) d -> (m t2) (j d)", j=2)
            cmbs = [
                sp.tile([128, 2 * D], F32, name=f"cmb{b}", tag=f"cmb{b}")
                for b in range(B_LOC)
            ]
            # interleave batches so both FMA chains progress concurrently
            for k in range(K):
                for b in range(B_LOC):
                    cmb = cmbs[b]
                    g = gp.tile([128, 2 * D], F32, tag="g")
                    nc.gpsimd.indirect_dma_start(
                        out=g[:],
                        out_offset=None,
                        in_=mem2,
                        in_offset=bass.IndirectOffsetOnAxis(
                            ap=ridu_all[b][:, k : k + 1], axis=0
                        ),
                    )
                    if k == 0:
                        nc.vector.tensor_scalar_mul(
                            cmb[:], g[:], w_all[b][:, k : k + 1]
                        )
                    else:
                        nc.vector.scalar_tensor_tensor(
                            out=cmb[:],
                            in0=g[:],
                            scalar=w_all[b][:, k : k + 1],
                            in1=cmb[:],
                            op0=mybir.AluOpType.mult,
                            op1=mybir.AluOpType.add,
                        )

            # ---- write combined into rows 1..T of the output ----
            # cmb[p, (j d)] holds t-rows t = 2*p + j.  (ACT ring: idle by now.)
            for b in range(B_LOC):
                nc.scalar.dma_start(
                    out=out[b, 1 : T + 1, :].rearrange("(p j) d -> p j d", j=2),
                    in_=cmbs[b][:].rearrange("p (j d) -> p j d", j=2),
                )

    nc.compile()
    return nc


def kernel(x, memory_tokens, memory_router):
    nc = build_program()
    in_maps = [
        {
            "x": np.ascontiguousarray(x[c * B_LOC : (c + 1) * B_LOC]),
            "mem": memory_tokens,
            "router": memory_router,
        }
        for c in range(N_CORES)
    ]
    res = run_bass_kernel_spmd(nc, in_maps, list(range(N_CORES)))
    return np.concatenate(
        [res.results[c]["out"] for c in range(N_CORES)], axis=0
    )
